# revision 7
# baseline (speedup 1.0000x reference)
"""Trainium2 Bass kernel for nn_CrossAttention_57698590654516.

Cross-attention: B=4, N=4096 (kv len), L=1024 (q len), C=1024, H=16 heads,
D=64. Sharding: 8 cores = (batch b = core//2) x (half the heads, core%2).
Each core computes, for its batch and its 8 heads:
  kT = Wk_part @ x.T          [512, N]   (T layout, head-major rows)
  v  = x @ Wv_part.T          [N, 512]   (+ interleaved ones col for denom)
  qT = Wq_part @ query.T      [512, L]
  scoresT_h = kT_h ops        [N, L] tilewise, exp via ScalarE (no max-sub:
                              scores are O(1) by construction)
  attn_outT_h[d, l] = sum_n v[n, d] * exp(s)/denom  (denom from ones column)
  y_partial = attn_outT.T @ WprojT_part   [L, C]
Host side: y[b] = y_partial[2b] + y_partial[2b+1] + bproj.

All matmuls run in float32r (TF32-like, ~1.5e-4 rel err per GEMM, full PE
rate at free-dim >= 256). Activations / accumulation stay fp32.
"""
import os
import sys

import numpy as np

try:
    import concourse.bass as bass  # noqa: F401
except ImportError:  # self-contained: find the repo in known locations
    for _p in ("/opt/trn_rl_repo", "/root/.axon_site/_ro/trn_rl_repo"):
        if os.path.isdir(_p) and _p not in sys.path:
            sys.path.insert(0, _p)
    import concourse.bass as bass  # noqa: F401

from contextlib import ExitStack

import concourse.tile as tile
from concourse import bacc, mybir
from concourse.bass_utils import run_bass_kernel_spmd

B, N, L, C, H = 4, 4096, 1024, 1024, 16
D = C // H  # 64
SCALE = 1.0 / float(np.sqrt(D))
P = 128
HPC = H // 2          # 8 heads per core
W = HPC * D           # 512 rows of k/v/q handled per core
F32 = mybir.dt.float32
F32R = mybir.dt.float32r
EXP = mybir.ActivationFunctionType.Exp

NCH = 256             # x chunk width (N elems per kv-pass chunk)
NCHUNKS = N // NCH    # 16


def build():
    nc = bacc.Bacc("TRN2", target_bir_lowering=False, debug=False, num_devices=8)
    # activations & weights arrive pre-transposed; declared float32r so they
    # feed matmuls directly (hardware uses the truncated mantissa).
    xT = nc.dram_tensor("xT", [C, N], F32R, kind="ExternalInput").ap()
    queryT = nc.dram_tensor("queryT", [C, L], F32R, kind="ExternalInput").ap()
    wkT = nc.dram_tensor("wkT", [C, W], F32R, kind="ExternalInput").ap()
    wvT = nc.dram_tensor("wvT", [C, W], F32R, kind="ExternalInput").ap()
    wqT = nc.dram_tensor("wqT", [C, W], F32R, kind="ExternalInput").ap()
    wprojT = nc.dram_tensor("wprojT", [W, C], F32R, kind="ExternalInput").ap()
    y = nc.dram_tensor("y", [L, C], F32, kind="ExternalOutput").ap()

    xT3 = xT.rearrange("(ko ki) n -> ki ko n", ki=P)          # [128, 8, N]
    queryT3 = queryT.rearrange("(ko ki) l -> ki ko l", ki=P)  # [128, 8, L]
    wkT3 = wkT.rearrange("(ko ki) m -> ki ko m", ki=P)        # [128, 8, 512]
    wvT3 = wvT.rearrange("(ko ki) m -> ki ko m", ki=P)
    wqT3 = wqT.rearrange("(ko ki) m -> ki ko m", ki=P)
    wprojT3 = wprojT.rearrange("(ko ki) c -> ki ko c", ki=P)  # [128, 4, 1024]

    with tile.TileContext(nc) as tc, ExitStack() as ctx:
        persist = ctx.enter_context(tc.tile_pool(name="persist", bufs=1))
        kT = persist.tile([P, 4, N], F32R, tag="kT")
        v520 = persist.tile([P, N // P, HPC, D + 1], F32R, tag="v520")
        onesrc = persist.tile([P, 1], F32, tag="onesrc")
        nc.any.memset(onesrc[:], 1.0)
        nc.vector.tensor_copy(v520[:, :, :, D],
                              onesrc[:, 0:1].to_broadcast([P, N // P, HPC]))

        # ---------------- KV pass 1: kT = Wk_part @ x.T ----------------
        with tc.tile_pool(name="kvw", bufs=1) as wp, \
             tc.tile_pool(name="kvstg", bufs=2) as stg, \
             tc.tile_pool(name="kvps", bufs=3, space="PSUM") as pp:
            wkr = wp.tile([P, 8, W], F32R, tag="wk")
            nc.sync.dma_start(wkr[:], wkT3[:])
            for c in range(NCHUNKS):
                xr = stg.tile([P, 8, NCH], F32R, tag="xs")
                nc.sync.dma_start(xr[:], xT3[:, :, c * NCH:(c + 1) * NCH])
                for m in range(4):
                    ps = pp.tile([P, NCH], F32, tag="kps")
                    for k in range(8):
                        nc.tensor.matmul(ps[:], wkr[:, k, m * P:(m + 1) * P],
                                         xr[:, k, :], start=(k == 0), stop=(k == 7))
                    nc.vector.tensor_copy(kT[:, m, c * NCH:(c + 1) * NCH], ps[:])

        # ---------------- KV pass 2: v = x @ Wv_part.T ----------------
        with tc.tile_pool(name="kvw2", bufs=1) as wp, \
             tc.tile_pool(name="kvstg2", bufs=2) as stg, \
             tc.tile_pool(name="kvps2", bufs=3, space="PSUM") as pp:
            wvr = wp.tile([P, 8, W], F32R, tag="wv")
            nc.sync.dma_start(wvr[:], wvT3[:])
            for c in range(NCHUNKS):
                xr = stg.tile([P, 8, NCH], F32R, tag="xs2")
                nc.sync.dma_start(xr[:], xT3[:, :, c * NCH:(c + 1) * NCH])
                for t in range(NCH // P):
                    ps = pp.tile([P, W], F32, tag="vps")
                    for k in range(8):
                        nc.tensor.matmul(ps[:], xr[:, k, t * P:(t + 1) * P],
                                         wvr[:, k, :], start=(k == 0), stop=(k == 7))
                    nt = c * (NCH // P) + t
                    nc.vector.tensor_copy(
                        v520[:, nt, :, 0:D],
                        ps[:].rearrange("p (h d) -> p h d", h=HPC))

        # ---------------- Q: qT = Wq_part @ query.T ----------------
        qT = persist.tile([P, 4, L], F32R, tag="qT")
        with tc.tile_pool(name="qw", bufs=1) as wp, \
             tc.tile_pool(name="qstg", bufs=2) as stg, \
             tc.tile_pool(name="qps", bufs=3, space="PSUM") as pp:
            wqr = wp.tile([P, 8, W], F32R, tag="wq")
            nc.sync.dma_start(wqr[:], wqT3[:])
            for lc in range(4):
                qr = stg.tile([P, 8, 256], F32R, tag="qs")
                nc.sync.dma_start(qr[:], queryT3[:, :, lc * 256:(lc + 1) * 256])
                for m in range(4):
                    ps = pp.tile([P, 256], F32, tag="qpsum")
                    for k in range(8):
                        nc.tensor.matmul(ps[:], wqr[:, k, m * P:(m + 1) * P],
                                         qr[:, k, :], start=(k == 0), stop=(k == 7))
                    nc.vector.tensor_copy(qT[:, m, lc * 256:(lc + 1) * 256], ps[:])

        # ---------------- Attention ----------------
        aoT = persist.tile([P, 4, L], F32R, tag="aoT")
        with tc.tile_pool(name="probs", bufs=3) as probs_pool, \
             tc.tile_pool(name="attsm", bufs=2) as small, \
             tc.tile_pool(name="spsum", bufs=3, space="PSUM") as spsum, \
             tc.tile_pool(name="apsum", bufs=2, space="PSUM") as apsum:
            for h in range(HPC):
                bp = D * (h % 2)      # 0 or 64: partition base within pair
                pr = h // 2           # pair index
                apts = [apsum.tile([D + 1, 512], F32, tag="apt",
                                   name=f"apt_{h}_{i}") for i in range(2)]
                for n in range(N // P):
                    spt = spsum.tile([P, 1024], F32, tag="spt")
                    for lc in range(2):
                        nc.tensor.matmul(
                            spt[:, lc * 512:(lc + 1) * 512],
                            kT[bp:bp + D, pr, n * P:(n + 1) * P],
                            qT[bp:bp + D, pr, lc * 512:(lc + 1) * 512],
                            start=True, stop=True)
                    pt = probs_pool.tile([P, 1024], F32R, tag="pt")
                    nc.scalar.activation(pt[:], spt[:], EXP, scale=SCALE)
                    for lc in range(2):
                        nc.tensor.matmul(
                            apts[lc][:], v520[:, n, h, :],
                            pt[:, lc * 512:(lc + 1) * 512],
                            start=(n == 0), stop=(n == N // P - 1))
                for lc in range(2):
                    apt = apts[lc]
                    r64t = small.tile([P, 512], F32, tag="r64")
                    nc.vector.reciprocal(r64t[D:D + 1, :], apt[D:D + 1, :])
                    rrow = small.tile([1, 512], F32, tag="rrow")
                    nc.sync.dma_start(rrow[:], r64t[D:D + 1, :])
                    rb = small.tile([D, 512], F32, tag="rb")
                    nc.gpsimd.partition_broadcast(rb[:], rrow[:])
                    dst = aoT[bp:bp + D, pr, lc * 512:(lc + 1) * 512]
                    if bp == 0:
                        nc.vector.tensor_mul(dst, apt[0:D, :], rb[:])
                    else:
                        tmp = small.tile([D, 512], F32R, tag="aotmp")
                        nc.vector.tensor_mul(tmp[:], apt[0:D, :], rb[:])
                        nc.sync.dma_start(dst, tmp[:])

        # -------- Proj: y_partial = attn_outT.T @ WprojT --------
        with tc.tile_pool(name="pw", bufs=1) as wp, \
             tc.tile_pool(name="ypool", bufs=3) as ypool, \
             tc.tile_pool(name="pps", bufs=3, space="PSUM") as pp:
            wpr = wp.tile([P, 4, C], F32R, tag="wp")
            nc.sync.dma_start(wpr[:], wprojT3[:])
            for l in range(L // P):
                for co in range(2):
                    ps = pp.tile([P, 512], F32, tag="yps")
                    for ci in range(4):
                        nc.tensor.matmul(ps[:], aoT[:, ci, l * P:(l + 1) * P],
                                         wpr[:, ci, co * 512:(co + 1) * 512],
                                         start=(ci == 0), stop=(ci == 3))
                    yt = ypool.tile([P, 512], F32, tag="yt")
                    nc.vector.tensor_copy(yt[:], ps[:])
                    nc.sync.dma_start(y[l * P:(l + 1) * P, co * 512:(co + 1) * 512],
                                      yt[:])
    nc.finalize()
    return nc


_NC_CACHE = {}


def get_nc():
    if "nc" not in _NC_CACHE:
        _NC_CACHE["nc"] = build()
    return _NC_CACHE["nc"]


def make_in_maps(x, query, Wq, Wkv, Wproj):
    x = np.asarray(x, dtype=np.float32)
    query = np.asarray(query, dtype=np.float32)
    Wq = np.asarray(Wq, dtype=np.float32)
    Wkv = np.asarray(Wkv, dtype=np.float32)
    Wproj = np.asarray(Wproj, dtype=np.float32)
    in_maps = []
    for core in range(8):
        b, half = core // 2, core % 2
        h0 = half * W  # 0 or 512: channel offset of this core's heads
        in_maps.append({
            "xT": np.ascontiguousarray(x[b].T),
            "queryT": np.ascontiguousarray(query[b].T),
            "wkT": np.ascontiguousarray(Wkv[h0:h0 + W, :].T),
            "wvT": np.ascontiguousarray(Wkv[C + h0:C + h0 + W, :].T),
            "wqT": np.ascontiguousarray(Wq[h0:h0 + W, :].T),
            "wprojT": np.ascontiguousarray(Wproj[:, h0:h0 + W].T),
        })
    return in_maps


def combine(results, bproj):
    y = np.zeros((B, L, C), np.float32)
    for core in range(8):
        y[core // 2] += results[core]["y"]
    y += np.asarray(bproj, dtype=np.float32)[None, None, :]
    return y


def kernel(x, query, Wq, Wkv, Wproj, bproj):
    nc = get_nc()
    in_maps = make_in_maps(x, query, Wq, Wkv, Wproj)
    res = run_bass_kernel_spmd(nc, in_maps, core_ids=list(range(8)))
    return combine(res.results, bproj)
